# revision 18
# baseline (speedup 1.0000x reference)
# Sliding-window causal multi-head attention with RoPE for Trainium2.
#
# Problem: B=4, T=2048, D=1024, H=16 heads, d_k=64, window=512.
#   q,k,v = x @ W{q,k,v}^T (split heads), RoPE(q,k), scores = q k^T / 8 with
#   mask 0 <= i-j <= 512, softmax, out = (attn @ v) concat-heads @ Wo^T.
#
# Sharding: 8 cores = (batch b in 0..3) x (sequence half). Each core computes
# output rows [half*1024, half*1024+1024) of batch b. It needs K/V for global
# rows [qbase-512, qbase+1024); for half 0 the first 512 rows don't exist and
# are zero-padded (host side), with a sums-correction term subtracted on-chip
# (padded keys contribute exp(0)=1 to the softmax denominator).
#
# On-chip pipeline (all matmuls bf16 with fp32 PSUM accumulation):
#   - x and W are cast f32->bf16 by SWDGE DMA into DRAM scratch, then
#     DMA-xbar-transposed into SBUF as x^T [m, t] and W^T [m, n] tiles.
#   - Q^T/K^T projections produce [128 = 2 heads x (evens|odds), t] tiles in
#     PSUM; RoPE is applied with host-provided cos/sin tables; the rotate-half
#     "swap" is a PE matmul with a permutation matrix.
#   - scores are computed transposed, S^T[k, q] = K Q^T, per (head, kv-block)
#     with the 5-block sliding window span; exp on ACT (scale=1/8 folded in);
#     boundary masks applied multiplicatively post-exp (gpsimd).
#   - PV uses lhsT = [V_h | ones] so the PSUM result holds both O^T (64 rows)
#     and the softmax denominator replicated (64 rows); normalization is
#     reciprocal_approx_fast + multiply, writing attnT [m', q] bf16 tiles that
#     feed the final Wo matmul directly.

import dataclasses
from contextlib import ExitStack

import numpy as np
import ml_dtypes

BF16 = ml_dtypes.bfloat16

B, T, D = 4, 2048, 1024
H, DK = 16, 64
WIN = 512
THETA = 10000.0
TQ, TKV = 1024, 1536
NBQ, NBKV = TQ // 128, TKV // 128  # 8, 12
NCHUNK = D // 128  # 8 contraction chunks
NPAIR = H // 2  # 8 head pairs

_CACHE = {}


def _pair_cols(ap2d, a, b, w):
    """From a [P, F] AP over contiguous cols, build an AP over cols
    {a..a+w} then {b..b+w} (2D free: outer count 2 step b-a)."""
    base = ap2d[:, a : a + w]
    return dataclasses.replace(base, ap=[base.ap[0], [b - a, 2], [1, w]])


def _build(debug_dumps=False):
    import concourse.bass as bass
    import concourse.bacc as bacc
    import concourse.mybir as mybir
    import concourse.tile as tile

    dt = mybir.dt
    F32, BF = dt.float32, dt.bfloat16
    AF = mybir.ActivationFunctionType
    OP = mybir.AluOpType

    nc = bacc.Bacc("TRN2", target_bir_lowering=False, debug=False, num_devices=8)

    # ---- DRAM I/O ----
    x_kv = nc.dram_tensor("x_kv", [TKV, D], F32, kind="ExternalInput").ap()
    w_in = {
        n: nc.dram_tensor(n, [D, D], F32, kind="ExternalInput").ap()
        for n in ("wq", "wk", "wv", "wo")
    }
    cos_in = nc.dram_tensor("cos_t", [128, TKV], F32, kind="ExternalInput").ap()
    sin_in = nc.dram_tensor("sin_t", [128, TKV], F32, kind="ExternalInput").ap()
    sign_in = nc.dram_tensor("sign_t", [128, 1], F32, kind="ExternalInput").ap()
    pswap_in = nc.dram_tensor("pswap", [128, 128], BF, kind="ExternalInput").ap()
    masks_in = nc.dram_tensor("masks", [128, 256], BF, kind="ExternalInput").ap()
    corr_in = nc.dram_tensor("corr", [2, TQ], BF, kind="ExternalInput").ap()
    cselE_in = nc.dram_tensor("cselE", [2, 128], BF, kind="ExternalInput").ap()
    cselO_in = nc.dram_tensor("cselO", [2, 128], BF, kind="ExternalInput").ap()
    out_d = nc.dram_tensor("out", [TQ, D], F32, kind="ExternalOutput").ap()

    # DRAM scratch for the cast-then-transpose path
    x_bf = nc.dram_tensor("x_bf", [TKV, D], BF, kind="Internal").ap()
    w_bf = {
        n: nc.dram_tensor(n + "_bf", [D, D], BF, kind="Internal").ap()
        for n in ("wq", "wk", "wv", "wo")
    }

    with ExitStack() as ctx:
        tc = ctx.enter_context(tile.TileContext(nc))

        big = ctx.enter_context(tc.tile_pool(name="big", bufs=1))
        wpool = ctx.enter_context(tc.tile_pool(name="wpool", bufs=2))
        ab = ctx.enter_context(tc.tile_pool(name="ab", bufs=4))
        epool = ctx.enter_context(tc.tile_pool(name="epool", bufs=8))
        rpool = ctx.enter_context(tc.tile_pool(name="rpool", bufs=2))
        stpool = ctx.enter_context(tc.tile_pool(name="stpool", bufs=2))
        pp = ctx.enter_context(tc.tile_pool(name="pp", bufs=2, space="PSUM"))
        scps = ctx.enter_context(tc.tile_pool(name="scps", bufs=2, space="PSUM"))
        pvps = ctx.enter_context(tc.tile_pool(name="pvps", bufs=1, space="PSUM"))

        # ---- persistent SBUF ----
        xT = big.tile([128, NCHUNK, TKV], BF)
        qT = big.tile([128, NPAIR, TQ], BF)
        kT = big.tile([128, NPAIR, TKV], BF)
        vS = big.tile([128, NBKV, 1024], BF)  # 16 heads x 64 cols
        onesS = big.tile([128, 64], BF)
        attnT = big.tile([128, NPAIR, TQ], BF)
        cosS = big.tile([128, TKV], F32)
        sinS = big.tile([128, TKV], F32)
        signS = big.tile([128, 1], F32)
        pswapS = big.tile([128, 128], BF)
        maskS = big.tile([128, 256], BF)
        corrS = big.tile([2, TQ], BF)
        cselES = big.tile([2, 128], BF)
        cselOS = big.tile([2, 128], BF)

        nc.sync.dma_start(out=cosS, in_=cos_in)
        nc.sync.dma_start(out=sinS, in_=sin_in)
        nc.sync.dma_start(out=signS, in_=sign_in)
        nc.sync.dma_start(out=pswapS, in_=pswap_in)
        nc.sync.dma_start(out=maskS, in_=masks_in)
        nc.sync.dma_start(out=corrS, in_=corr_in)
        nc.sync.dma_start(out=cselES, in_=cselE_in)
        nc.sync.dma_start(out=cselOS, in_=cselO_in)
        nc.vector.memset(onesS, 1.0)

        # ---- x: cast f32->bf16 (DRAM->DRAM) then xbar-transpose into SBUF ----
        for s in range(4):
            cs = slice(s * 256, s * 256 + 256)
            nc.gpsimd.dma_start(out=x_bf[:, cs], in_=x_kv[:, cs])
        for c in range(NCHUNK):
            nc.sync.dma_start(
                out=xT[:, c, :], in_=x_bf[:, c * 128 : c * 128 + 128], transpose=True
            )

        def prep_w(name, eo_permute=False):
            # Cast f32->bf16 into DRAM scratch, then xbar-transpose to SBUF.
            # For Wq/Wk the rows (head output dims) are permuted during the
            # cast so each head's 64 dims land as (evens, odds) — RoPE's
            # rotate-half then only needs 32-row group swaps, and the
            # projection lhsT slices stay contiguous.
            wt = wpool.tile([128, NCHUNK, D], BF, tag="wT")
            for s in range(2):
                cs = slice(s * 512, s * 512 + 512)
                if eo_permute:
                    # source row n = 64g+2j+e -> dest row 64g+32e+j (g=2b+h).
                    # One DMA per parity e keeps the APs at 3 dims.
                    R = D
                    for e in range(2):
                        src = w_in[name][:, cs]
                        src = dataclasses.replace(
                            src,
                            offset=src.offset + e * R,
                            ap=[[64 * R, 16], [2 * R, 32], [1, 512]],
                        )
                        dst = w_bf[name][:, cs]
                        dst = dataclasses.replace(
                            dst,
                            offset=dst.offset + 32 * e * R,
                            ap=[[64 * R, 16], [R, 32], [1, 512]],
                        )
                        nc.gpsimd.dma_start(out=dst, in_=src)
                else:
                    nc.gpsimd.dma_start(out=w_bf[name][:, cs], in_=w_in[name][:, cs])
            for c in range(NCHUNK):
                nc.sync.dma_start(
                    out=wt[:, c, :],
                    in_=w_bf[name][:, c * 128 : c * 128 + 128],
                    transpose=True,
                )
            return wt

        def qk_proj(wt, dest, nchunks, coff):
            # dest: [128, NPAIR, nchunks*512] roped bf16; x^T cols [coff, coff+nchunks*512)
            for r in range(NPAIR):
                for tch in range(nchunks):
                    tsl = slice(coff + tch * 512, coff + tch * 512 + 512)
                    osl = slice(tch * 512, tch * 512 + 512)
                    ps = pp.tile([128, 512], F32, tag="pp")
                    for c in range(NCHUNK):
                        nc.tensor.matmul(
                            ps,
                            wt[:, c, r * 128 : r * 128 + 128],
                            xT[:, c, tsl],
                            start=(c == 0),
                            stop=(c == NCHUNK - 1),
                        )
                    w1 = ab.tile([128, 512], BF, tag="w1")
                    t2 = ab.tile([128, 512], BF, tag="t2")
                    nc.vector.tensor_mul(w1, ps, sinS[:, tsl])
                    nc.vector.tensor_mul(t2, ps, cosS[:, tsl])
                    us = pp.tile([128, 512], F32, tag="pp")
                    nc.tensor.matmul(us, pswapS, w1, start=True, stop=True)
                    # rope = swap(P*sin) * sign + P*cos
                    nc.vector.scalar_tensor_tensor(
                        out=dest[:, r, osl],
                        in0=us,
                        scalar=signS[:, 0:1],
                        in1=t2,
                        op0=OP.mult,
                        op1=OP.add,
                    )

        wqT = prep_w("wq", eo_permute=True)
        wkT = prep_w("wk", eo_permute=True)
        qk_proj(wqT, qT, 2, 512)  # queries = kv rows 512..1536
        qk_proj(wkT, kT, 3, 0)
        wvT = prep_w("wv")

        # ---- V projection: natural [t, n] layout ----
        for tt in range(NBKV):
            for nh in range(2):
                ps = pp.tile([128, 512], F32, tag="pp")
                for c in range(NCHUNK):
                    nc.tensor.matmul(
                        ps,
                        xT[:, c, tt * 128 : tt * 128 + 128],
                        wvT[:, c, nh * 512 : nh * 512 + 512],
                        start=(c == 0),
                        stop=(c == NCHUNK - 1),
                    )
                nc.scalar.copy(out=vS[:, tt, nh * 512 : nh * 512 + 512], in_=ps)

        woT = prep_w("wo")

        # ---- attention ----
        # kv block b serves q blocks g in [max(0,b-4), min(b,7)]
        for p in range(NPAIR):
            for sub in range(2):  # 0: head 2p (rows 0:64), 1: head 2p+1 (rows 64:128)
                h = 2 * p + sub
                rows = slice(64 * sub, 64 * sub + 64)
                pv = pvps.tile([128, TQ], F32, tag="pv")
                e_tiles = {}
                for b in range(NBKV):
                    glo, ghi = max(0, b - 4), min(b, NBQ - 1)
                    span = (ghi - glo + 1) * 128
                    q0 = glo * 128
                    sc = scps.tile([128, 640], F32, tag="sc")
                    for c0 in range(0, span, 512):
                        c1 = min(c0 + 512, span)
                        nc.tensor.matmul(
                            sc[:, c0:c1],
                            kT[rows, p, b * 128 : b * 128 + 128],
                            qT[rows, p, q0 + c0 : q0 + c1],
                            start=True,
                            stop=True,
                        )
                    et = epool.tile([128, 640], BF, tag="et")
                    nc.scalar.activation(
                        out=et[:, 0:span], in_=sc[:, 0:span], func=AF.Exp, scale=0.125
                    )
                    # boundary masks (multiplicative, post-exp)
                    has_diag = b >= 4  # q block g=b-4 at span cols 0:128
                    has_triu = b <= NBQ - 1  # q block g=b at last 128 cols
                    if has_diag and has_triu:
                        sel = _pair_cols(et[:, 0:640], 0, span - 128, 128)
                        nc.gpsimd.tensor_mul(sel, sel, maskS[:, 0:256])
                    elif has_diag:
                        nc.gpsimd.tensor_mul(
                            et[:, 0:128], et[:, 0:128], maskS[:, 0:128]
                        )
                    else:
                        sl = slice(span - 128, span)
                        nc.gpsimd.tensor_mul(et[:, sl], et[:, sl], maskS[:, 128:256])
                    e_tiles[b] = (et, q0, span)

                # PV accumulation; bank A first-touch = b=3 (covers q cols 0:512
                # exactly), bank B first-touch = b=7's [512:1024) part.
                # start=True must cover its bank's full extent before any
                # accumulation there: bank A starter = b=3 (spans [0,512)
                # exactly), bank B starter = b=8 (spans [512,1024) exactly).
                # Emit all bank-A segments (in E-availability order), then
                # bank B starting at b=8.
                orderA = [(3, 0, 512)]
                orderB = []
                for b in range(NBKV):
                    glo, ghi = max(0, b - 4), min(b, NBQ - 1)
                    qa, qb = glo * 128, (ghi + 1) * 128
                    if qa < 512 and b != 3:
                        orderA.append((b, qa, min(qb, 512)))
                    if qb > 512 and b != 8:
                        orderB.append((b, max(qa, 512), qb))
                order = orderA + [(8, 512, 1024)] + orderB
                started = set()
                # O rows and sums rows: even heads put O low / sums high,
                # odd heads the reverse, so attnT chunk p is [head 2p; head
                # 2p+1] and every later elementwise op stays lane-aligned.
                olo, rlo = (0, 64) if sub == 0 else (64, 0)
                for b, s0, s1 in order:
                    et, q0, span = e_tiles[b]
                    bank = s0 // 512
                    st = bank not in started
                    rhs = et[:, s0 - q0 : s1 - q0]
                    nc.tensor.matmul(
                        pv[olo : olo + 64, s0:s1],
                        vS[:, b, 64 * h : 64 * h + 64],
                        rhs,
                        start=st,
                        stop=False,
                        skip_group_check=True,
                        tile_position=(0, olo),
                    )
                    nc.tensor.matmul(
                        pv[rlo : rlo + 64, s0:s1],
                        onesS,
                        rhs,
                        start=st,
                        stop=False,
                        skip_group_check=True,
                        tile_position=(0, rlo),
                    )
                    started.add(bank)
                # softmax-denominator correction for the zero-padded keys
                csel = cselES if sub == 0 else cselOS
                for bank in range(2):
                    nc.tensor.matmul(
                        pv[:, bank * 512 : bank * 512 + 512],
                        csel,
                        corrS[:, bank * 512 : bank * 512 + 512],
                        start=False,
                        stop=True,
                        skip_group_check=True,
                    )
                # normalize: attnT[rows_h] = O / sums.
                # reciprocal_approx_fast (custom DVE op) is broken at
                # partition base 64 on HW, so always run it at base 0.
                rec = rpool.tile([128, TQ], F32, tag="rec")
                lo, hi = slice(0, 64), slice(64, 128)
                if sub == 0:  # O low, sums high
                    nc.scalar.copy(out=rec[hi, :], in_=pv[hi, :])
                    nc.gpsimd.tensor_copy(rec[lo, :], rec[hi, :])
                    nc.vector.reciprocal_approx_fast(out=rec[lo, :], in_=rec[lo, :])
                    nc.vector.tensor_mul(attnT[lo, p, :], pv[lo, :], rec[lo, :])
                else:  # O high, sums low
                    nc.vector.reciprocal_approx_fast(out=rec[lo, :], in_=pv[lo, :])
                    nc.gpsimd.tensor_copy(rec[hi, :], rec[lo, :])
                    nc.vector.tensor_mul(attnT[hi, p, :], pv[hi, :], rec[hi, :])

        if debug_dumps:
            for nm, tl, sh in (
                ("d_xT", xT, [128, NCHUNK * TKV]),
                ("d_qT", qT, [128, NPAIR * TQ]),
                ("d_kT", kT, [128, NPAIR * TKV]),
                ("d_vS", vS, [128, NBKV * 1024]),
                ("d_attnT", attnT, [128, NPAIR * TQ]),
                ("d_wqT", wqT, [128, NCHUNK * D]),
                ("d_woT", woT, [128, NCHUNK * D]),
            ):
                dd = nc.dram_tensor(nm, sh, BF, kind="ExternalOutput").ap()
                nc.sync.dma_start(out=dd, in_=tl)

        # ---- output projection ----
        for qt in range(NBQ):
            st = stpool.tile([128, D], F32, tag="st")
            for nh in range(2):
                ps = pp.tile([128, 512], F32, tag="pp")
                for c in range(NPAIR):
                    nc.tensor.matmul(
                        ps,
                        attnT[:, c, qt * 128 : qt * 128 + 128],
                        woT[:, c, nh * 512 : nh * 512 + 512],
                        start=(c == 0),
                        stop=(c == NPAIR - 1),
                    )
                nc.scalar.copy(out=st[:, nh * 512 : nh * 512 + 512], in_=ps)
            nc.sync.dma_start(out=out_d[qt * 128 : qt * 128 + 128, :], in_=st)

    nc.compile()
    return nc


def _host_inputs(x, token_positions, Wq, Wk, Wv, Wo):
    x = np.ascontiguousarray(np.asarray(x, dtype=np.float32))
    pos = np.asarray(token_positions).astype(np.int64)
    ws = {
        "wq": np.ascontiguousarray(np.asarray(Wq, np.float32)),
        "wk": np.ascontiguousarray(np.asarray(Wk, np.float32)),
        "wv": np.ascontiguousarray(np.asarray(Wv, np.float32)),
        "wo": np.ascontiguousarray(np.asarray(Wo, np.float32)),
    }
    invf = THETA ** (-np.arange(32, dtype=np.float64) * 2.0 / DK)
    sign = np.tile(np.repeat(np.float32([-1, 1]), 32), 2).reshape(128, 1)
    perm = np.r_[32:64, 0:32, 96:128, 64:96]
    P = np.zeros((128, 128), np.float32)
    P[np.arange(128), perm] = 1.0
    pswapT = np.ascontiguousarray(P.T).astype(BF16)
    cidx = np.arange(128)[:, None]
    ridx = np.arange(128)[None, :]
    m_diag = (ridx >= cidx).astype(BF16)
    m_triu = (ridx <= cidx).astype(BF16)
    masks = np.ascontiguousarray(np.concatenate([m_diag, m_triu], axis=1))

    in_maps = []
    for core in range(8):
        b, half = divmod(core, 2)
        qbase = half * TQ
        if half == 0:
            xkv = np.concatenate([np.zeros((WIN, D), np.float32), x[b, :TQ]], axis=0)
        else:
            xkv = np.ascontiguousarray(x[b, T - TKV :])
        j = qbase - WIN + np.arange(TKV)
        jv = np.clip(j, 0, T - 1)
        posv = np.where((j >= 0) & (j < T), pos[jv], 0).astype(np.float64)
        ang = invf[:, None] * posv[None, :]  # [32, TKV]
        cos_t = np.ascontiguousarray(np.tile(np.cos(ang), (4, 1)).astype(np.float32))
        sin_t = np.ascontiguousarray(np.tile(np.sin(ang), (4, 1)).astype(np.float32))
        gi = qbase + np.arange(TQ)
        corrv = np.maximum(0, WIN - gi).astype(np.float32) if half == 0 else np.zeros(TQ, np.float32)
        corrA = np.minimum(corrv, 256.0)
        corr = np.ascontiguousarray(np.stack([corrA, corrv - corrA]).astype(BF16))
        cselE = np.zeros((2, 128), np.float32)
        cselE[:, 64:] = -1.0
        cselO = np.zeros((2, 128), np.float32)
        cselO[:, :64] = -1.0
        in_maps.append(
            {
                "x_kv": xkv,
                **ws,
                "cos_t": cos_t,
                "sin_t": sin_t,
                "sign_t": sign,
                "pswap": pswapT,
                "masks": masks,
                "corr": corr,
                "cselE": cselE.astype(BF16),
                "cselO": cselO.astype(BF16),
            }
        )
    return in_maps


def _get_nc():
    if "nc" not in _CACHE:
        _CACHE["nc"] = _build()
    return _CACHE["nc"]


def kernel(x, token_positions, Wq, Wk, Wv, Wo, _trace=False):
    from concourse.bass_utils import run_bass_kernel_spmd

    nc = _get_nc()
    in_maps = _host_inputs(x, token_positions, Wq, Wk, Wv, Wo)
    res = run_bass_kernel_spmd(nc, in_maps, core_ids=list(range(8)), trace=_trace)
    _CACHE["last_result"] = res
    out = np.zeros((B, T, D), np.float32)
    for core in range(8):
        b, half = divmod(core, 2)
        out[b, half * TQ : half * TQ + TQ] = res.results[core]["out"]
    return out


# revision 25
# speedup vs baseline: 1.0682x; 1.0682x over previous
# Sliding-window causal multi-head attention with RoPE for Trainium2.
#
# Problem: B=4, T=2048, D=1024, H=16 heads, d_k=64, window=512.
#   q,k,v = x @ W{q,k,v}^T (split heads), RoPE(q,k), scores = q k^T / 8 with
#   mask 0 <= i-j <= 512, softmax, out = (attn @ v) concat-heads @ Wo^T.
#
# Sharding: 8 cores = (batch b in 0..3) x (sequence half). Each core computes
# output rows [half*1024, half*1024+1024) of batch b. It needs K/V for global
# rows [qbase-512, qbase+1024); for half 0 the first 512 rows don't exist and
# are zero-padded (host side), with a sums-correction term subtracted on-chip
# (padded keys contribute exp(0)=1 to the softmax denominator).
#
# On-chip pipeline (all matmuls bf16 with fp32 PSUM accumulation):
#   - x and W are cast f32->bf16 by SWDGE DMA into DRAM scratch, then
#     DMA-xbar-transposed into SBUF as x^T [m, t] and W^T [m, n] tiles.
#   - Q^T/K^T projections produce [128 = 2 heads x (evens|odds), t] tiles in
#     PSUM; RoPE is applied with host-provided cos/sin tables; the rotate-half
#     "swap" is a PE matmul with a permutation matrix.
#   - scores are computed transposed, S^T[k, q] = K Q^T, per (head, kv-block)
#     with the 5-block sliding window span; exp on ACT (scale=1/8 folded in);
#     boundary masks applied multiplicatively post-exp (gpsimd).
#   - PV uses lhsT = [V_h | ones] so the PSUM result holds both O^T (64 rows)
#     and the softmax denominator replicated (64 rows); normalization is
#     reciprocal_approx_fast + multiply, writing attnT [m', q] bf16 tiles that
#     feed the final Wo matmul directly.

import dataclasses
from contextlib import ExitStack

import numpy as np
import ml_dtypes

BF16 = ml_dtypes.bfloat16

B, T, D = 4, 2048, 1024
H, DK = 16, 64
WIN = 512
THETA = 10000.0
TQ, TKV = 1024, 1536
NBQ, NBKV = TQ // 128, TKV // 128  # 8, 12
NCHUNK = D // 128  # 8 contraction chunks
NPAIR = H // 2  # 8 head pairs

_CACHE = {}


def _pair_cols(ap2d, a, b, w):
    """From a [P, F] AP over contiguous cols, build an AP over cols
    {a..a+w} then {b..b+w} (2D free: outer count 2 step b-a)."""
    base = ap2d[:, a : a + w]
    return dataclasses.replace(base, ap=[base.ap[0], [b - a, 2], [1, w]])


def _build(debug_dumps=False):
    import concourse.bass as bass
    import concourse.bacc as bacc
    import concourse.mybir as mybir
    import concourse.tile as tile

    dt = mybir.dt
    F32, BF = dt.float32, dt.bfloat16
    AF = mybir.ActivationFunctionType
    OP = mybir.AluOpType

    nc = bacc.Bacc("TRN2", target_bir_lowering=False, debug=False, num_devices=8)

    # ---- DRAM I/O ----
    x_kv = nc.dram_tensor("x_kv", [TKV, D], F32, kind="ExternalInput").ap()
    w_in = {
        n: nc.dram_tensor(n, [D, D], F32, kind="ExternalInput").ap()
        for n in ("wq", "wk", "wv", "wo")
    }
    cos_in = nc.dram_tensor("cos_t", [128, TKV], BF, kind="ExternalInput").ap()
    sin_in = nc.dram_tensor("sin_t", [128, TKV], BF, kind="ExternalInput").ap()
    sign_in = nc.dram_tensor("sign_t", [128, 1], F32, kind="ExternalInput").ap()
    pswap_in = nc.dram_tensor("pswap", [128, 128], BF, kind="ExternalInput").ap()
    masks_in = nc.dram_tensor("masks", [128, 256], BF, kind="ExternalInput").ap()
    corr_in = nc.dram_tensor("corr", [2, TQ], BF, kind="ExternalInput").ap()
    cselE_in = nc.dram_tensor("cselE", [2, 128], BF, kind="ExternalInput").ap()
    cselO_in = nc.dram_tensor("cselO", [2, 128], BF, kind="ExternalInput").ap()
    out_d = nc.dram_tensor("out", [TQ, D], F32, kind="ExternalOutput").ap()

    # DRAM scratch for the cast-then-transpose path
    x_bf = nc.dram_tensor("x_bf", [TKV, D], BF, kind="Internal").ap()
    w_bf = {
        n: nc.dram_tensor(n + "_bf", [D, D], BF, kind="Internal").ap()
        for n in ("wq", "wk", "wv", "wo")
    }

    with ExitStack() as ctx:
        tc = ctx.enter_context(tile.TileContext(nc))

        big = ctx.enter_context(tc.tile_pool(name="big", bufs=1))
        wpool = ctx.enter_context(tc.tile_pool(name="wpool", bufs=2))
        ab = ctx.enter_context(tc.tile_pool(name="ab", bufs=4))
        epool = ctx.enter_context(tc.tile_pool(name="epool", bufs=8))
        rpool = ctx.enter_context(tc.tile_pool(name="rpool", bufs=2))
        stpool = ctx.enter_context(tc.tile_pool(name="stpool", bufs=2))
        pp = ctx.enter_context(tc.tile_pool(name="pp", bufs=2, space="PSUM"))
        scps = ctx.enter_context(tc.tile_pool(name="scps", bufs=2, space="PSUM"))
        pvps = ctx.enter_context(tc.tile_pool(name="pvps", bufs=1, space="PSUM"))

        # ---- persistent SBUF ----
        xT = big.tile([128, NCHUNK, TKV], BF)
        qT = big.tile([128, NPAIR, TQ], BF)
        kT = big.tile([128, NPAIR, TKV], BF)
        vS = big.tile([128, NBKV, 1024], BF)  # 16 heads x 64 cols
        onesS = big.tile([128, 64], BF)
        attnT = big.tile([128, NPAIR, TQ], BF)
        cosS = big.tile([128, TKV], BF)
        sinS = big.tile([128, TKV], BF)
        signS = big.tile([128, 1], F32)
        pswapS = big.tile([128, 128], BF)
        maskS = big.tile([128, 256], BF)
        corrS = big.tile([2, TQ], BF)
        cselES = big.tile([2, 128], BF)
        cselOS = big.tile([2, 128], BF)

        nc.sync.dma_start(out=cosS, in_=cos_in)
        nc.sync.dma_start(out=sinS, in_=sin_in)
        nc.sync.dma_start(out=signS, in_=sign_in)
        nc.sync.dma_start(out=pswapS, in_=pswap_in)
        nc.sync.dma_start(out=maskS, in_=masks_in)
        nc.sync.dma_start(out=corrS, in_=corr_in)
        nc.sync.dma_start(out=cselES, in_=cselE_in)
        nc.sync.dma_start(out=cselOS, in_=cselO_in)
        nc.vector.memset(onesS, 1.0)

        # ---- x: cast f32->bf16 (DRAM->DRAM) then xbar-transpose into SBUF.
        # One whole-matrix 3D-output transpose per column half; the two
        # halves go to different HWDGE engines so they overlap.
        for half, eng in ((0, nc.sync), (1, nc.scalar)):
            cs = slice(half * 512, half * 512 + 512)
            nc.gpsimd.dma_start(out=x_bf[:, cs], in_=x_kv[:, cs])
            eng.dma_start(out=xT[:, 4 * half : 4 * half + 4, :], in_=x_bf[:, cs], transpose=True)

        _weng = [nc.sync, nc.scalar]

        def prep_w(name, eo_permute=False):
            # Cast f32->bf16 into DRAM scratch, then one whole-matrix
            # xbar-transpose to SBUF. For Wq/Wk the rows (head output dims)
            # are permuted during the cast so each head's 64 dims land as
            # (evens, odds) — RoPE's rotate-half then only needs 32-row
            # group swaps, and the projection lhsT slices stay contiguous.
            wt = wpool.tile([128, NCHUNK, D], BF, tag="wT")
            if eo_permute:
                # source row n = 64g+2j+e -> dest row 64g+32e+j (g=2b+h).
                # One DMA per parity e keeps the APs at 3 dims.
                R = D
                for e in range(2):
                    src = w_in[name]
                    src = dataclasses.replace(
                        src,
                        offset=src.offset + e * R,
                        ap=[[64 * R, 16], [2 * R, 32], [1, D]],
                    )
                    dst = w_bf[name]
                    dst = dataclasses.replace(
                        dst,
                        offset=dst.offset + 32 * e * R,
                        ap=[[64 * R, 16], [R, 32], [1, D]],
                    )
                    nc.gpsimd.dma_start(out=dst, in_=src)
            else:
                nc.gpsimd.dma_start(out=w_bf[name], in_=w_in[name])
            eng = _weng.pop(0)
            _weng.append(eng)
            eng.dma_start(out=wt, in_=w_bf[name], transpose=True)
            return wt

        def qk_proj(wt, dest, nchunks, coff):
            # dest: [128, NPAIR, nchunks*512] roped bf16; x^T cols [coff, coff+nchunks*512)
            for r in range(NPAIR):
                for tch in range(nchunks):
                    tsl = slice(coff + tch * 512, coff + tch * 512 + 512)
                    osl = slice(tch * 512, tch * 512 + 512)
                    ps = pp.tile([128, 512], F32, tag="pp")
                    for c in range(NCHUNK):
                        nc.tensor.matmul(
                            ps,
                            wt[:, c, r * 128 : r * 128 + 128],
                            xT[:, c, tsl],
                            start=(c == 0),
                            stop=(c == NCHUNK - 1),
                        )
                    # evacuate psum to bf16 once (ACT), then both RoPE
                    # muls run in DVE 2x mode on all-bf16 SBUF operands
                    pb = ab.tile([128, 512], BF, tag="pb")
                    nc.scalar.copy(out=pb, in_=ps)
                    w1 = ab.tile([128, 512], BF, tag="w1")
                    t2 = ab.tile([128, 512], BF, tag="t2")
                    nc.vector.tensor_mul(w1, pb, sinS[:, tsl])
                    nc.vector.tensor_mul(t2, pb, cosS[:, tsl])
                    us = pp.tile([128, 512], F32, tag="pp")
                    nc.tensor.matmul(us, pswapS, w1, start=True, stop=True)
                    # rope = swap(P*sin) * sign + P*cos
                    nc.vector.scalar_tensor_tensor(
                        out=dest[:, r, osl],
                        in0=us,
                        scalar=signS[:, 0:1],
                        in1=t2,
                        op0=OP.mult,
                        op1=OP.add,
                    )

        wqT = prep_w("wq", eo_permute=True)
        wkT = prep_w("wk", eo_permute=True)
        qk_proj(wqT, qT, 2, 512)  # queries = kv rows 512..1536
        qk_proj(wkT, kT, 3, 0)
        wvT = prep_w("wv")

        # ---- V projection: natural [t, n] layout ----
        for tt in range(NBKV):
            for nh in range(2):
                ps = pp.tile([128, 512], F32, tag="pp")
                for c in range(NCHUNK):
                    nc.tensor.matmul(
                        ps,
                        xT[:, c, tt * 128 : tt * 128 + 128],
                        wvT[:, c, nh * 512 : nh * 512 + 512],
                        start=(c == 0),
                        stop=(c == NCHUNK - 1),
                    )
                nc.scalar.copy(out=vS[:, tt, nh * 512 : nh * 512 + 512], in_=ps)

        woT = prep_w("wo")

        # ---- attention ----
        # kv block b serves q blocks g in [max(0,b-4), min(b,7)]
        for p in range(NPAIR):
            for sub in range(2):  # 0: head 2p (rows 0:64), 1: head 2p+1 (rows 64:128)
                h = 2 * p + sub
                rows = slice(64 * sub, 64 * sub + 64)
                pv = pvps.tile([128, TQ], F32, tag="pv")
                e_tiles = {}
                for b in range(NBKV):
                    glo, ghi = max(0, b - 4), min(b, NBQ - 1)
                    span = (ghi - glo + 1) * 128
                    q0 = glo * 128
                    sc = scps.tile([128, 640], F32, tag="sc")
                    for c0 in range(0, span, 512):
                        c1 = min(c0 + 512, span)
                        nc.tensor.matmul(
                            sc[:, c0:c1],
                            kT[rows, p, b * 128 : b * 128 + 128],
                            qT[rows, p, q0 + c0 : q0 + c1],
                            start=True,
                            stop=True,
                        )
                    et = epool.tile([128, 640], BF, tag="et")
                    nc.scalar.activation(
                        out=et[:, 0:span], in_=sc[:, 0:span], func=AF.Exp, scale=0.125
                    )
                    # boundary masks (multiplicative, post-exp); alternate
                    # between DVE and GpSimd to balance engine load
                    meng = nc.vector if (h + b) % 2 == 0 else nc.gpsimd
                    has_diag = b >= 4  # q block g=b-4 at span cols 0:128
                    has_triu = b <= NBQ - 1  # q block g=b at last 128 cols
                    if has_diag and has_triu:
                        sel = _pair_cols(et[:, 0:640], 0, span - 128, 128)
                        meng.tensor_mul(sel, sel, maskS[:, 0:256])
                    elif has_diag:
                        meng.tensor_mul(et[:, 0:128], et[:, 0:128], maskS[:, 0:128])
                    else:
                        sl = slice(span - 128, span)
                        meng.tensor_mul(et[:, sl], et[:, sl], maskS[:, 128:256])
                    e_tiles[b] = (et, q0, span)

                # PV accumulation; bank A first-touch = b=3 (covers q cols 0:512
                # exactly), bank B first-touch = b=7's [512:1024) part.
                # start=True must cover its bank's full extent before any
                # accumulation there: bank A starter = b=3 (spans [0,512)
                # exactly), bank B starter = b=8 (spans [512,1024) exactly).
                # Emit all bank-A segments (in E-availability order), then
                # bank B starting at b=8.
                orderA = [(3, 0, 512)]
                orderB = []
                for b in range(NBKV):
                    glo, ghi = max(0, b - 4), min(b, NBQ - 1)
                    qa, qb = glo * 128, (ghi + 1) * 128
                    if qa < 512 and b != 3:
                        orderA.append((b, qa, min(qb, 512)))
                    if qb > 512 and b != 8:
                        orderB.append((b, max(qa, 512), qb))
                order = orderA + [(8, 512, 1024)] + orderB
                started = set()
                # O rows and sums rows: even heads put O low / sums high,
                # odd heads the reverse, so attnT chunk p is [head 2p; head
                # 2p+1] and every later elementwise op stays lane-aligned.
                olo, rlo = (0, 64) if sub == 0 else (64, 0)
                for b, s0, s1 in order:
                    et, q0, span = e_tiles[b]
                    bank = s0 // 512
                    st = bank not in started
                    rhs = et[:, s0 - q0 : s1 - q0]
                    nc.tensor.matmul(
                        pv[olo : olo + 64, s0:s1],
                        vS[:, b, 64 * h : 64 * h + 64],
                        rhs,
                        start=st,
                        stop=False,
                        skip_group_check=True,
                        tile_position=(0, olo),
                    )
                    nc.tensor.matmul(
                        pv[rlo : rlo + 64, s0:s1],
                        onesS,
                        rhs,
                        start=st,
                        stop=False,
                        skip_group_check=True,
                        tile_position=(0, rlo),
                    )
                    started.add(bank)
                # softmax-denominator correction for the zero-padded keys
                csel = cselES if sub == 0 else cselOS
                for bank in range(2):
                    nc.tensor.matmul(
                        pv[:, bank * 512 : bank * 512 + 512],
                        csel,
                        corrS[:, bank * 512 : bank * 512 + 512],
                        start=False,
                        stop=True,
                        skip_group_check=True,
                    )
                # normalize: attnT[rows_h] = O / sums.
                # reciprocal_approx_fast (custom DVE op) is broken at
                # partition base 64 on HW, so always run it at base 0;
                # cross-partition-base operands on plain DVE ops are fine.
                rec = rpool.tile([128, TQ], F32, tag="rec")
                lo, hi = slice(0, 64), slice(64, 128)
                if sub == 0:  # O low, sums high
                    nc.vector.tensor_copy(rec[lo, :], pv[hi, :])
                    nc.vector.reciprocal_approx_fast(out=rec[lo, :], in_=rec[lo, :])
                    nc.vector.tensor_mul(attnT[lo, p, :], pv[lo, :], rec[lo, :])
                else:  # O high, sums low
                    nc.vector.reciprocal_approx_fast(out=rec[lo, :], in_=pv[lo, :])
                    nc.vector.tensor_mul(attnT[hi, p, :], pv[hi, :], rec[lo, :])

        if debug_dumps:
            for nm, tl, sh in (
                ("d_xT", xT, [128, NCHUNK * TKV]),
                ("d_qT", qT, [128, NPAIR * TQ]),
                ("d_kT", kT, [128, NPAIR * TKV]),
                ("d_vS", vS, [128, NBKV * 1024]),
                ("d_attnT", attnT, [128, NPAIR * TQ]),
                ("d_wqT", wqT, [128, NCHUNK * D]),
                ("d_woT", woT, [128, NCHUNK * D]),
            ):
                dd = nc.dram_tensor(nm, sh, BF, kind="ExternalOutput").ap()
                nc.sync.dma_start(out=dd, in_=tl)

        # ---- output projection ----
        for qt in range(NBQ):
            st = stpool.tile([128, D], F32, tag="st")
            for nh in range(2):
                ps = pp.tile([128, 512], F32, tag="pp")
                for c in range(NPAIR):
                    nc.tensor.matmul(
                        ps,
                        attnT[:, c, qt * 128 : qt * 128 + 128],
                        woT[:, c, nh * 512 : nh * 512 + 512],
                        start=(c == 0),
                        stop=(c == NPAIR - 1),
                    )
                nc.scalar.copy(out=st[:, nh * 512 : nh * 512 + 512], in_=ps)
            nc.sync.dma_start(out=out_d[qt * 128 : qt * 128 + 128, :], in_=st)

    nc.compile()
    return nc


def _host_inputs(x, token_positions, Wq, Wk, Wv, Wo):
    x = np.ascontiguousarray(np.asarray(x, dtype=np.float32))
    pos = np.asarray(token_positions).astype(np.int64)
    ws = {
        "wq": np.ascontiguousarray(np.asarray(Wq, np.float32)),
        "wk": np.ascontiguousarray(np.asarray(Wk, np.float32)),
        "wv": np.ascontiguousarray(np.asarray(Wv, np.float32)),
        "wo": np.ascontiguousarray(np.asarray(Wo, np.float32)),
    }
    invf = THETA ** (-np.arange(32, dtype=np.float64) * 2.0 / DK)
    sign = np.tile(np.repeat(np.float32([-1, 1]), 32), 2).reshape(128, 1)
    perm = np.r_[32:64, 0:32, 96:128, 64:96]
    P = np.zeros((128, 128), np.float32)
    P[np.arange(128), perm] = 1.0
    pswapT = np.ascontiguousarray(P.T).astype(BF16)
    cidx = np.arange(128)[:, None]
    ridx = np.arange(128)[None, :]
    m_diag = (ridx >= cidx).astype(BF16)
    m_triu = (ridx <= cidx).astype(BF16)
    masks = np.ascontiguousarray(np.concatenate([m_diag, m_triu], axis=1))

    in_maps = []
    for core in range(8):
        b, half = divmod(core, 2)
        qbase = half * TQ
        if half == 0:
            xkv = np.concatenate([np.zeros((WIN, D), np.float32), x[b, :TQ]], axis=0)
        else:
            xkv = np.ascontiguousarray(x[b, T - TKV :])
        j = qbase - WIN + np.arange(TKV)
        jv = np.clip(j, 0, T - 1)
        posv = np.where((j >= 0) & (j < T), pos[jv], 0).astype(np.float64)
        ang = invf[:, None] * posv[None, :]  # [32, TKV]
        cos_t = np.ascontiguousarray(np.tile(np.cos(ang), (4, 1)).astype(BF16))
        sin_t = np.ascontiguousarray(np.tile(np.sin(ang), (4, 1)).astype(BF16))
        gi = qbase + np.arange(TQ)
        corrv = np.maximum(0, WIN - gi).astype(np.float32) if half == 0 else np.zeros(TQ, np.float32)
        corrA = np.minimum(corrv, 256.0)
        corr = np.ascontiguousarray(np.stack([corrA, corrv - corrA]).astype(BF16))
        cselE = np.zeros((2, 128), np.float32)
        cselE[:, 64:] = -1.0
        cselO = np.zeros((2, 128), np.float32)
        cselO[:, :64] = -1.0
        in_maps.append(
            {
                "x_kv": xkv,
                **ws,
                "cos_t": cos_t,
                "sin_t": sin_t,
                "sign_t": sign,
                "pswap": pswapT,
                "masks": masks,
                "corr": corr,
                "cselE": cselE.astype(BF16),
                "cselO": cselO.astype(BF16),
            }
        )
    return in_maps


def _get_nc():
    if "nc" not in _CACHE:
        _CACHE["nc"] = _build()
    return _CACHE["nc"]


def kernel(x, token_positions, Wq, Wk, Wv, Wo, _trace=False):
    from concourse.bass_utils import run_bass_kernel_spmd

    nc = _get_nc()
    in_maps = _host_inputs(x, token_positions, Wq, Wk, Wv, Wo)
    res = run_bass_kernel_spmd(nc, in_maps, core_ids=list(range(8)), trace=_trace)
    _CACHE["last_result"] = res
    out = np.zeros((B, T, D), np.float32)
    for core in range(8):
        b, half = divmod(core, 2)
        out[b, half * TQ : half * TQ + TQ] = res.results[core]["out"]
    return out
